# revision 20
# baseline (speedup 1.0000x reference)
"""Trainium2 Bass kernel for nn_BornFoward: 200-step leapfrog wave recurrence.

Math (validated against the jax reference):
  - coef = (dt*BGF/dx)^2 is 0.2025 in the interior square [25:167)^2 of the
    192x192 grid and ~4.4e-13 in the outer absorbing ring; rf is EXACTLY zero
    outside the central 96x96 window (pad region has X==1 -> 1-X^2==0).
  - Therefore the recurrence restricted to the 142x142 interior with zero
    Dirichlet boundary and constant coef reproduces the reference to ~1e-9.
  - p_new = 2*p1 - p0 + C*lap4(p1) + rf*d2(P0),  meas = p_new at 32 pixels.

Sharding: 16 independent recurrences (B=2 x NR=8) -> channel r per core,
both batch fields per core.

Kernel structure (12 matmuls per step for both fields -- the PE floor for
this stencil given one-rhs-window-per-matmul and 128-partition APs;
matmul cost ~ out free size only):
  - PSUM per (field, parity): one bank [128, 2, 146] fp32 holding BOTH
    71-row chunks side by side in the free dim (block 0 = domain rows
    [0,71), block 1 = [71,142)).
  - 4 tap matmuls per field do the y-stencil for BOTH chunks at once:
    lhsT = w*I [71,128], rhs = state tile [71, 2, 142] at col offsets
    +-1, +-2, out [128, 2, 142] (N=284).  The first (+1 tap) opens the
    whole bank (start=True); the 2 band matmuls close their blocks.
  - 2 band matmuls per field (N=142) do the x-stencil via a Toeplitz
    banded lhsT over K=121 rhs partitions: 71 state rows + 48 G-annex
    identity rows (source term rides the contraction for free) + 2
    seam-halo rows (the other chunk's boundary rows).  Receiver
    selection (aug) columns ride as out partitions 96..127.
  - state tiles rotate through THREE slots (cur -> prev -> spare) so the
    per-step G DMA can be issued 3 steps ahead of use: with the HWDGE
    fixed cost (625ns) + DGE delay + sem propagation (~2.6us total) a
    2-step lookahead cannot land in time at a ~1.2us step.
  - copyback p_new = PSUM - p0 is one 3D DVE op per field writing the
    spare slot (in1 = prev slot, 3 distinct APs).  Seam halos live at
    partitions 96,97 (32-aligned so GPSIMD may write them): block0's
    halo copy (source partition 0) runs on Pool; block1's (source
    partition 69, Pool-illegal) on Act (f0) / DVE (f1).  All tiny
    SBUF->SBUF copies of just-written state rows.
  - measurements: one Act copy per field moves PSUM aug rows [32,2,142]
    into a 16-slot rolling fp16 buffer (field 0 on partitions 0..31,
    field 1 on 32..63), DMA'd to DRAM in 8-slot blocks on SP/HWDGE;
    the host picks each receiver's (block, column).
  - G (rf*d2) is DMA'd as ONE transfer per step for both fields straight
    into the annex partitions 71..120 (1168B contiguous per partition;
    the two halo partitions 96,97 get zeros, overwritten later by the
    seam copies of that slot's step).
"""
import sys
import os
import numpy as np
from contextlib import ExitStack

sys.path.insert(0, "/opt/trn_rl_repo")

# ---- problem constants (hardcoded; kernel.py must be self-contained) ----
NX = 192
NT = 200
dtime = 0.3
nm, sR = 32, 70
bg = 1.5
LO, HI = 25, 167            # interior rows/cols [LO, HI) -> D = 142
D = HI - LO
CLO, CHI = 48, 144          # central rf-support window (96 wide)
CW = CHI - CLO
COFF = CLO - LO             # 23: central window offset inside domain
C = (dtime * bg / 1.0) ** 2  # 0.2025
K = 71                      # chunk rows (2 row-chunks of 71 = 142)
KA = 121                    # rhs partitions: 71 state + 48 G + 2 halo
HALO = 96                   # halo partitions 96,97 (32-aligned for GPSIMD)
SEG = 2 + D + 2             # 146: per-chunk col run with 2-col guards
NRR = 8
BB = 2
NMEAS = nm

_thetas = 2 * np.pi * np.arange(nm) / nm
_MX = (NX / 2 + sR * np.cos(_thetas)).astype(int)
_MY = (NX / 2 + sR * np.sin(_thetas)).astype(int)
# receivers sorted chunk-0-first so each chunk's aug rows are contiguous
_GROW = _MX - LO                       # domain row per receiver
_ORDER = np.argsort(_GROW >= K, kind="stable")
_N0 = int(np.sum(_GROW < K))


def _gpart(s):
    """Annex partition for G row s (annex split around the halo pair)."""
    return 71 + s if s < 25 else 73 + s


_prog_cache = {}


def _build_band_consts():
    """Host-side constant matrices for the matmuls (numpy float16).

    Stencil band f(d): d=0: -60C/12+2 (2*p1 folded in), |d|=1: 16C/12,
    |d|=2: -C/12.
    """
    f1 = 16.0 * C / 12.0
    f2 = -C / 12.0
    fband = {0: -60.0 * C / 12.0 + 2.0, 1: f1, -1: f1, 2: f2, -2: f2}

    BD0 = np.zeros((KA, 128), np.float16)
    BD1 = np.zeros((KA, 128), np.float16)
    for k in range(K):
        for m in range(max(0, k - 2), min(K, k + 3)):
            BD0[k, m] = fband[m - k]
            BD1[k, m] = fband[m - k]
    for s in range(48):
        BD0[_gpart(s), COFF + s] = 1.0   # G0: out domain row 23+s
        BD1[_gpart(s), s] = 1.0          # G1: out domain row 71+s
    # halo parts 96,97: BD0 halo = domain rows 71,72 -> out rows 69,70
    BD0[96, 69] = f2
    BD0[96, 70] = f1
    BD0[97, 70] = f2
    # block1's seam (domain rows 69,70 -> its rows 0,1) travels via a
    # small cross matmul XB instead of a halo: every engine needs
    # 32-aligned partition bases, so no engine can copy rows 69,70.
    XB = np.zeros((K, 128), np.float16)
    XB[69, 0] = f2
    XB[70, 0] = f1
    XB[70, 1] = f2
    # aug selection: receiver i (sorted) -> out partition 96+pos
    for pos, i in enumerate(_ORDER):
        g = _GROW[i]
        if g < K:
            BD0[g, 96 + pos] = 1.0
        else:
            BD1[g - K, 96 + pos] = 1.0

    TAP1 = np.zeros((K, 128), np.float16)
    TAP2 = np.zeros((K, 128), np.float16)
    for q in range(K):
        TAP1[q, q] = f1
        TAP2[q, q] = f2
    return {"BD0": BD0, "BD1": BD1, "TAP1": TAP1, "TAP2": TAP2, "XB": XB}


def _build_program(nt=NT, reps=1):
    import concourse.bacc as bacc
    import concourse.tile as tile
    import concourse.mybir as mybir

    dt = mybir.dt
    nc = bacc.Bacc("TRN2", target_bir_lowering=False)

    # G packed per step as [50 parts, field, block, 146]: one contiguous
    # 1168B run per partition covering annex+halo partitions 71..120
    # (halo rows 25,26 of this layout are zeros)
    G_d = nc.dram_tensor("G", (NT, 50, BB, 2, SEG), dt.float16,
                         kind="ExternalInput")
    CSHAPES = [("BD0", (KA, 128)), ("BD1", (KA, 128)),
               ("TAP1", (K, 128)), ("TAP2", (K, 128)), ("XB", (K, 128))]
    CWID = sum(s[1][1] for s in CSHAPES)
    CP_d = nc.dram_tensor("CPACK", (KA, CWID), dt.float16,
                          kind="ExternalInput")
    NBLK = (nt + 7) // 8
    MEAS_d = nc.dram_tensor("MEAS", (BB, NMEAS, NBLK, 8, 2, D), dt.float16,
                            kind="ExternalOutput")

    with tile.TileContext(nc) as tc, ExitStack() as ctx:
        def sbuf(name, shape, dty):
            return ctx.enter_context(nc.sbuf_tensor(name, shape, dty))

        # 3 rotating state slots, both fields in one tensor:
        # [partitions, field, block, cols]
        TT = [sbuf(f"TT{s}", [KA, BB, 2, SEG], dt.float16) for s in range(3)]
        cpack = sbuf("cpack", [KA, CWID], dt.float16)
        ct, _co = {}, 0
        for n, shp in CSHAPES:
            ct[n] = cpack[0:shp[0], _co:_co + shp[1]]
            _co += shp[1]
        # 16-slot rolling meas staging: field f on partitions 32f..32f+31
        msb = sbuf("msb", [2 * NMEAS, 16, 2, D], dt.float16)

        # one PSUM tensor per (field, parity): both blocks side by side
        PS = [[ctx.enter_context(
                   nc.psum_tensor(f"PS{f}{p}", [128, 2, SEG], dt.float32))
               for p in range(2)] for f in range(BB)]

        for s in range(3):
            (nc.vector if s == 0 else nc.gpsimd).memset(TT[s][:], 0.0)
        nc.scalar.memzero(msb[:])
        nc.sync.dma_start(cpack[:], CP_d[:])

        def g_dma(q, slot, eng=None):
            """DMA G[q] (both fields) into a slot's annex+halo partitions."""
            (eng or nc.sync).dma_start(
                TT[slot][71:121, 0:2, 0:2, 0:SEG], G_d[q])

        mmkw = dict(skip_group_check=True)
        cur, prev, spare = 0, 1, 2
        for rep in range(reps):
          if rep > 0:
            for s in range(3):
                (nc.vector if s == 0 else nc.gpsimd).memset(TT[s][:], 0.0)
          # G[0]=G[1]=0 and the state starts at zero, so p^(1)=p^(2)=0:
          # steps 0 and 1 are no-ops. Start at j=2 (the host zero-fills
          # meas[0]; meas[1]=p^(2) is staged -- correctly zero -- by
          # step 2's aug columns).
          for j in range(2, nt):
              p = j % 2
              if j == 2:
                  # cur-at-2 = cur, cur-at-3 = spare, cur-at-4 = prev
                  g_dma(2, cur, eng=nc.scalar)
                  g_dma(3, spare, eng=nc.scalar)
                  g_dma(4, prev, eng=nc.scalar)

              for f in range(BB):
                  O = PS[f][p]
                  tc_ = TT[cur]
                  mm = nc.tensor.matmul
                  # 4 y-tap matmuls covering BOTH chunks (N=284); the
                  # first opens the whole bank.  Taps depend only on the
                  # copyback, so the seam/G-consuming band matmuls sit a
                  # 4-matmul-deep window later.
                  mm(O[0:128, 0:2, 2:2 + D], ct["TAP1"],
                     tc_[0:K, f, 0:2, 3:3 + D],
                     start=True, stop=False, **mmkw)
                  mm(O[0:128, 0:2, 2:2 + D], ct["TAP1"],
                     tc_[0:K, f, 0:2, 1:1 + D],
                     start=False, stop=False, **mmkw)
                  mm(O[0:128, 0:2, 2:2 + D], ct["TAP2"],
                     tc_[0:K, f, 0:2, 4:4 + D],
                     start=False, stop=False, **mmkw)
                  mm(O[0:128, 0:2, 2:2 + D], ct["TAP2"],
                     tc_[0:K, f, 0:2, 0:D],
                     start=False, stop=False, **mmkw)
                  # cross matmul: block1's seam contribution (domain
                  # rows 69,70 -> its rows 0,1) straight from block0's rhs
                  mm(O[0:128, 1, 2:2 + D], ct["XB"],
                     tc_[0:K, f, 0, 2:2 + D],
                     start=False, stop=False, **mmkw)
                  # 2 band matmuls (x-stencil + 2*p1 + G + halo + aug)
                  mm(O[0:128, 0, 2:2 + D], ct["BD0"],
                     tc_[0:KA, f, 0, 2:2 + D],
                     start=False, stop=True, **mmkw)
                  mm(O[0:128, 1, 2:2 + D], ct["BD1"],
                     tc_[0:KA, f, 1, 2:2 + D],
                     start=False, stop=True, **mmkw)

                  # copyback: p_new = PSUM - p0 into the spare slot
                  nc.vector.tensor_tensor(
                      out=TT[spare][0:K, f, 0:2, 2:2 + D],
                      in0=O[0:K, 0:2, 2:2 + D],
                      in1=TT[prev][0:K, f, 0:2, 2:2 + D],
                      op=mybir.AluOpType.subtract)
                  # block0's seam halo (partitions 96,97, 32-aligned
                  # so Pool may write it) <- just-written block1 rows 0,1
                  nc.gpsimd.tensor_copy(
                      out=TT[spare][HALO:HALO + 2, f, 0, 2:2 + D],
                      in_=TT[spare][0:2, f, 1, 2:2 + D])

                  # stage meas slot j-1 = p^j (aug rows are copies of p1)
                  if rep == reps - 1:
                      nc.scalar.copy(
                          msb[32 * f:32 * f + NMEAS, (j - 1) % 16],
                          O[96:96 + NMEAS, 0:2, 2:2 + D])

              # one merged G DMA, 3 steps ahead (lands mid step j+2)
              if j + 3 < nt:
                  g_dma(j + 3, cur)

              # stream finished 8-slot blocks, fields staggered by 2 steps
              if rep == reps - 1:
                  if j % 8 == 0:
                      b = j // 8 - 1
                      nc.sync.dma_start(
                          MEAS_d[0][:, b],
                          msb[0:NMEAS, (b % 2) * 8:(b % 2) * 8 + 8])
                  if j % 8 == 2 and j >= 10:
                      b = (j - 2) // 8 - 1
                      nc.sync.dma_start(
                          MEAS_d[1][:, b],
                          msb[NMEAS:2 * NMEAS, (b % 2) * 8:(b % 2) * 8 + 8])

              cur, prev, spare = spare, cur, prev

        # post-loop: stage slot nt-1 = p^(nt) via the aug columns of one
        # extra band pair per field (annex/band rows write junk state rows
        # that are never read; aug rows are a pure copy of p^(nt))
        for f in range(BB):
            O = PS[f][nt % 2]
            nc.tensor.matmul(O[0:128, 0, 2:2 + D], ct["BD0"],
                             TT[cur][0:KA, f, 0, 2:2 + D],
                             start=True, stop=False, skip_group_check=True)
            nc.tensor.matmul(O[0:128, 1, 2:2 + D], ct["BD1"],
                             TT[cur][0:KA, f, 1, 2:2 + D],
                             start=True, stop=True, skip_group_check=True)
        for f in range(BB):
            nc.scalar.copy(msb[32 * f:32 * f + NMEAS, (nt - 1) % 16],
                           PS[f][nt % 2][96:96 + NMEAS, 0:2, 2:2 + D])
        # flush the final (partial) block for both fields
        b = (nt - 1) // 8
        for f in range(BB):
            nc.sync.dma_start(
                MEAS_d[f][:, b],
                msb[32 * f:32 * f + NMEAS, (b % 2) * 8:(b % 2) * 8 + 8])

    nc.compile()
    return nc


def kernel(x, P0):
    x = np.asarray(x, dtype=np.float32)
    P0 = np.asarray(P0, dtype=np.float32)
    from concourse.bass_utils import run_bass_kernel_spmd

    if "prog" not in _prog_cache:
        _prog_cache["prog"] = _build_program()
    nc = _prog_cache["prog"]

    cb = _build_band_consts()
    order = ["BD0", "BD1", "TAP1", "TAP2", "XB"]
    wid = sum(cb[n].shape[1] for n in order)
    cp = np.zeros((KA, wid), np.float16)
    co = 0
    for n in order:
        a = cb[n]
        cp[0:a.shape[0], co:co + a.shape[1]] = a
        co += a.shape[1]
    consts = {"CPACK": cp}

    xx = bg / x[:, 0]
    rf = (1.0 - xx * xx).astype(np.float32)           # (B, 96, 96)
    P0c = P0[0, :, :, CLO:CHI, CLO:CHI]               # (NR, NT, 96, 96)
    d2 = np.zeros_like(P0c)
    d2[:, 2:] = P0c[:, 2:] - 2.0 * P0c[:, 1:-1] + P0c[:, :-2]

    grows = np.array([_gpart(s) - 71 for s in range(48)])
    in_maps = []
    for r in range(NRR):
        Gc = (rf[None, :, :, :] * d2[r][:, None, :, :]).astype(np.float16)
        G = np.zeros((NT, 50, BB, 2, SEG), np.float16)
        # fancy index on axis 1 puts that axis first in the result
        G[:, grows, :, 0, 2 + COFF:2 + COFF + CW] = \
            Gc[:, :, 0:48, :].transpose(2, 0, 1, 3)
        G[:, grows, :, 1, 2 + COFF:2 + COFF + CW] = \
            Gc[:, :, 48:96, :].transpose(2, 0, 1, 3)
        m = dict(consts)
        m["G"] = G
        in_maps.append(m)

    trace = bool(int(os.environ.get("KERNEL_TRACE", "0")))
    res = run_bass_kernel_spmd(nc, in_maps, core_ids=list(range(NRR)),
                               trace=trace)
    _prog_cache["last_result"] = res

    ry = _MY - LO
    out = np.zeros((BB, NRR, NMEAS, NT), np.float32)
    nblk = (NT + 7) // 8
    for r in range(NRR):
        Ms = res.results[r]["MEAS"].reshape(BB, NMEAS, nblk * 8, 2, D)
        for f in range(BB):
            for pos, i in enumerate(_ORDER):
                c = 0 if _GROW[i] < K else 1
                out[f, r, i] = Ms[f, pos, :NT, c, ry[i]].astype(np.float32)
            out[f, r, :, 0] = 0.0
    return out


# revision 21
# speedup vs baseline: 1.0321x; 1.0321x over previous
"""Trainium2 Bass kernel for nn_BornFoward: 200-step leapfrog wave recurrence.

Math (validated against the jax reference):
  - coef = (dt*BGF/dx)^2 is 0.2025 in the interior square [25:167)^2 of the
    192x192 grid and ~4.4e-13 in the outer absorbing ring; rf is EXACTLY zero
    outside the central 96x96 window (pad region has X==1 -> 1-X^2==0).
  - Therefore the recurrence restricted to the 142x142 interior with zero
    Dirichlet boundary and constant coef reproduces the reference to ~1e-9.
  - p_new = 2*p1 - p0 + C*lap4(p1) + rf*d2(P0),  meas = p_new at 32 pixels.

Sharding: 16 independent recurrences (B=2 x NR=8) -> channel r per core,
both batch fields per core.

Kernel structure (12 matmuls per step for both fields -- the PE floor for
this stencil given one-rhs-window-per-matmul and 128-partition APs;
matmul cost ~ out free size only):
  - PSUM per (field, parity): one bank [128, 2, 146] fp32 holding BOTH
    71-row chunks side by side in the free dim (block 0 = domain rows
    [0,71), block 1 = [71,142)).
  - 4 tap matmuls per field do the y-stencil for BOTH chunks at once:
    lhsT = w*I [71,128], rhs = state tile [71, 2, 142] at col offsets
    +-1, +-2, out [128, 2, 142] (N=284).  The first (+1 tap) opens the
    whole bank (start=True); the 2 band matmuls close their blocks.
  - 2 band matmuls per field (N=142) do the x-stencil via a Toeplitz
    banded lhsT over K=121 rhs partitions: 71 state rows + 48 G-annex
    identity rows (source term rides the contraction for free) + 2
    seam-halo rows (the other chunk's boundary rows).  Receiver
    selection (aug) columns ride as out partitions 96..127.
  - state tiles rotate through THREE slots (cur -> prev -> spare) so the
    per-step G DMA can be issued 3 steps ahead of use: with the HWDGE
    fixed cost (625ns) + DGE delay + sem propagation (~2.6us total) a
    2-step lookahead cannot land in time at a ~1.2us step.
  - copyback p_new = PSUM - p0 is one 3D DVE op per field writing the
    spare slot (in1 = prev slot, 3 distinct APs).  Seam halos live at
    partitions 96,97 (32-aligned so GPSIMD may write them): block0's
    halo copy (source partition 0) runs on Pool; block1's (source
    partition 69, Pool-illegal) on Act (f0) / DVE (f1).  All tiny
    SBUF->SBUF copies of just-written state rows.
  - measurements: one Act copy per field moves PSUM aug rows [32,2,142]
    into a 16-slot rolling fp16 buffer (field 0 on partitions 0..31,
    field 1 on 32..63), DMA'd to DRAM in 8-slot blocks on SP/HWDGE;
    the host picks each receiver's (block, column).
  - G (rf*d2) is DMA'd as ONE transfer per step for both fields straight
    into the annex partitions 71..120 (1168B contiguous per partition;
    the two halo partitions 96,97 get zeros, overwritten later by the
    seam copies of that slot's step).
"""
import sys
import os
import numpy as np
from contextlib import ExitStack

sys.path.insert(0, "/opt/trn_rl_repo")

# ---- problem constants (hardcoded; kernel.py must be self-contained) ----
NX = 192
NT = 200
dtime = 0.3
nm, sR = 32, 70
bg = 1.5
LO, HI = 25, 167            # interior rows/cols [LO, HI) -> D = 142
D = HI - LO
CLO, CHI = 48, 144          # central rf-support window (96 wide)
CW = CHI - CLO
COFF = CLO - LO             # 23: central window offset inside domain
C = (dtime * bg / 1.0) ** 2  # 0.2025
K = 71                      # chunk rows (2 row-chunks of 71 = 142)
KA = 121                    # rhs partitions: 71 state + 48 G + 2 halo
HALO = 96                   # halo partitions 96,97 (32-aligned for GPSIMD)
SEG = 2 + D + 2             # 146: per-chunk col run with 2-col guards
NRR = 8
BB = 2
NMEAS = nm

_thetas = 2 * np.pi * np.arange(nm) / nm
_MX = (NX / 2 + sR * np.cos(_thetas)).astype(int)
_MY = (NX / 2 + sR * np.sin(_thetas)).astype(int)
# receivers sorted chunk-0-first so each chunk's aug rows are contiguous
_GROW = _MX - LO                       # domain row per receiver
_ORDER = np.argsort(_GROW >= K, kind="stable")
_N0 = int(np.sum(_GROW < K))


def _gpart(s):
    """Annex partition for G row s (annex split around the halo pair)."""
    return 71 + s if s < 25 else 73 + s


_prog_cache = {}


def _build_band_consts():
    """Host-side constant matrices for the matmuls (numpy float16).

    Stencil band f(d): d=0: -60C/12+2 (2*p1 folded in), |d|=1: 16C/12,
    |d|=2: -C/12.
    """
    f1 = 16.0 * C / 12.0
    f2 = -C / 12.0
    fband = {0: -60.0 * C / 12.0 + 2.0, 1: f1, -1: f1, 2: f2, -2: f2}

    BD0 = np.zeros((KA, 128), np.float16)
    BD1 = np.zeros((KA, 128), np.float16)
    for k in range(K):
        for m in range(max(0, k - 2), min(K, k + 3)):
            BD0[k, m] = fband[m - k]
            BD1[k, m] = fband[m - k]
    for s in range(48):
        BD0[_gpart(s), COFF + s] = 1.0   # G0: out domain row 23+s
        BD1[_gpart(s), s] = 1.0          # G1: out domain row 71+s
    # halo parts 96,97: BD0 halo = domain rows 71,72 -> out rows 69,70
    BD0[96, 69] = f2
    BD0[96, 70] = f1
    BD0[97, 70] = f2
    # block1's seam (domain rows 69,70 -> its rows 0,1) travels via a
    # small cross matmul XB instead of a halo: every engine needs
    # 32-aligned partition bases, so no engine can copy rows 69,70.
    XB = np.zeros((K, 128), np.float16)
    XB[69, 0] = f2
    XB[70, 0] = f1
    XB[70, 1] = f2
    # aug selection: receiver i (sorted) -> out partition 96+pos
    for pos, i in enumerate(_ORDER):
        g = _GROW[i]
        if g < K:
            BD0[g, 96 + pos] = 1.0
        else:
            BD1[g - K, 96 + pos] = 1.0

    TAP1 = np.zeros((K, 128), np.float16)
    TAP2 = np.zeros((K, 128), np.float16)
    for q in range(K):
        TAP1[q, q] = f1
        TAP2[q, q] = f2
    return {"BD0": BD0, "BD1": BD1, "TAP1": TAP1, "TAP2": TAP2, "XB": XB}


def _build_program(nt=NT, reps=1):
    import concourse.bacc as bacc
    import concourse.tile as tile
    import concourse.mybir as mybir

    dt = mybir.dt
    nc = bacc.Bacc("TRN2", target_bir_lowering=False)

    # G packed per step as [50 parts, field, block, 146]: one contiguous
    # 1168B run per partition covering annex+halo partitions 71..120
    # (halo rows 25,26 of this layout are zeros)
    G_d = nc.dram_tensor("G", (NT, 50, BB, 2, SEG), dt.float16,
                         kind="ExternalInput")
    CSHAPES = [("BD0", (KA, 128)), ("BD1", (KA, 128)),
               ("TAP1", (K, 128)), ("TAP2", (K, 128)), ("XB", (K, 128))]
    CWID = sum(s[1][1] for s in CSHAPES)
    CP_d = nc.dram_tensor("CPACK", (KA, CWID), dt.float16,
                          kind="ExternalInput")
    NBLK = (nt + 7) // 8
    MEAS_d = nc.dram_tensor("MEAS", (BB, NMEAS, NBLK, 8, 2, D), dt.float16,
                            kind="ExternalOutput")

    with tile.TileContext(nc) as tc, ExitStack() as ctx:
        def sbuf(name, shape, dty):
            return ctx.enter_context(nc.sbuf_tensor(name, shape, dty))

        # 3 rotating state slots, both fields in one tensor:
        # [partitions, field, block, cols]
        TT = [sbuf(f"TT{s}", [KA, BB, 2, SEG], dt.float16) for s in range(3)]
        cpack = sbuf("cpack", [KA, CWID], dt.float16)
        ct, _co = {}, 0
        for n, shp in CSHAPES:
            ct[n] = cpack[0:shp[0], _co:_co + shp[1]]
            _co += shp[1]
        # 16-slot rolling meas staging: field f on partitions 32f..32f+31
        msb = sbuf("msb", [2 * NMEAS, 16, 2, D], dt.float16)

        # one PSUM tensor per (field, parity): both blocks side by side
        PS = [[ctx.enter_context(
                   nc.psum_tensor(f"PS{f}{p}", [128, 2, SEG], dt.float32))
               for p in range(2)] for f in range(BB)]

        for s in range(3):
            (nc.vector if s == 0 else nc.gpsimd).memset(TT[s][:], 0.0)
        nc.scalar.memzero(msb[:])
        nc.sync.dma_start(cpack[:], CP_d[:])

        def g_dma(q, slot, eng=None):
            """DMA G[q] (both fields) into a slot's annex+halo partitions."""
            (eng or nc.sync).dma_start(
                TT[slot][71:121, 0:2, 0:2, 0:SEG], G_d[q])

        mmkw = dict(skip_group_check=True)
        cur, prev, spare = 0, 1, 2
        for rep in range(reps):
          if rep > 0:
            for s in range(3):
                (nc.vector if s == 0 else nc.gpsimd).memset(TT[s][:], 0.0)
          # G[0]=G[1]=0 and the state starts at zero, so p^(1)=p^(2)=0:
          # steps 0 and 1 are no-ops. Start at j=2 (the host zero-fills
          # meas[0]; meas[1]=p^(2) is staged -- correctly zero -- by
          # step 2's aug columns).
          for j in range(2, nt):
              p = j % 2
              if j == 2:
                  # cur-at-2 = cur, cur-at-3 = spare, cur-at-4 = prev
                  g_dma(2, cur, eng=nc.scalar)
                  g_dma(3, spare, eng=nc.scalar)
                  g_dma(4, prev, eng=nc.scalar)

              for f in range(BB):
                  O = PS[f][p]
                  tc_ = TT[cur]
                  mm = nc.tensor.matmul
                  # block1 first: its taps, the cross matmul carrying the
                  # seam from block0's rhs, then its band (stop).  Block1
                  # finishing at ~35% of the field block lets its copyback
                  # launch early so the chain (last mm -> copyback ->
                  # next-step taps) stays under the PE-bound period.
                  mm(O[0:128, 1, 2:2 + D], ct["TAP1"],
                     tc_[0:K, f, 1, 3:3 + D],
                     start=True, stop=False, **mmkw)
                  mm(O[0:128, 1, 2:2 + D], ct["TAP1"],
                     tc_[0:K, f, 1, 1:1 + D],
                     start=False, stop=False, **mmkw)
                  mm(O[0:128, 1, 2:2 + D], ct["TAP2"],
                     tc_[0:K, f, 1, 4:4 + D],
                     start=False, stop=False, **mmkw)
                  mm(O[0:128, 1, 2:2 + D], ct["TAP2"],
                     tc_[0:K, f, 1, 0:D],
                     start=False, stop=False, **mmkw)
                  mm(O[0:128, 1, 2:2 + D], ct["XB"],
                     tc_[0:K, f, 0, 2:2 + D],
                     start=False, stop=False, **mmkw)
                  mm(O[0:128, 1, 2:2 + D], ct["BD1"],
                     tc_[0:KA, f, 1, 2:2 + D],
                     start=False, stop=True, **mmkw)
                  # block0: taps then band (which consumes the seam halo
                  # written last step -- placed late to give it slack)
                  mm(O[0:128, 0, 2:2 + D], ct["TAP1"],
                     tc_[0:K, f, 0, 3:3 + D],
                     start=True, stop=False, **mmkw)
                  mm(O[0:128, 0, 2:2 + D], ct["TAP1"],
                     tc_[0:K, f, 0, 1:1 + D],
                     start=False, stop=False, **mmkw)
                  mm(O[0:128, 0, 2:2 + D], ct["TAP2"],
                     tc_[0:K, f, 0, 4:4 + D],
                     start=False, stop=False, **mmkw)
                  mm(O[0:128, 0, 2:2 + D], ct["TAP2"],
                     tc_[0:K, f, 0, 0:D],
                     start=False, stop=False, **mmkw)
                  mm(O[0:128, 0, 2:2 + D], ct["BD0"],
                     tc_[0:KA, f, 0, 2:2 + D],
                     start=False, stop=True, **mmkw)

                  # copybacks p_new = PSUM - p0, split per block so
                  # block1's launches mid-field-block
                  nc.vector.tensor_tensor(
                      out=TT[spare][0:K, f, 1, 2:2 + D],
                      in0=O[0:K, 1, 2:2 + D],
                      in1=TT[prev][0:K, f, 1, 2:2 + D],
                      op=mybir.AluOpType.subtract)
                  nc.vector.tensor_tensor(
                      out=TT[spare][0:K, f, 0, 2:2 + D],
                      in0=O[0:K, 0, 2:2 + D],
                      in1=TT[prev][0:K, f, 0, 2:2 + D],
                      op=mybir.AluOpType.subtract)
                  # block0's seam halo (partitions 96,97, 32-aligned
                  # so Pool may write it) <- just-written block1 rows 0,1
                  nc.gpsimd.tensor_copy(
                      out=TT[spare][HALO:HALO + 2, f, 0, 2:2 + D],
                      in_=TT[spare][0:2, f, 1, 2:2 + D])

                  # stage meas slot j-1 = p^j (aug rows are copies of p1)
                  if rep == reps - 1:
                      nc.scalar.copy(
                          msb[32 * f:32 * f + NMEAS, (j - 1) % 16],
                          O[96:96 + NMEAS, 0:2, 2:2 + D])

              # one merged G DMA, 3 steps ahead (lands mid step j+2)
              if j + 3 < nt:
                  g_dma(j + 3, cur)

              # stream finished 8-slot blocks, fields staggered by 2 steps
              if rep == reps - 1:
                  if j % 8 == 0:
                      b = j // 8 - 1
                      nc.sync.dma_start(
                          MEAS_d[0][:, b],
                          msb[0:NMEAS, (b % 2) * 8:(b % 2) * 8 + 8])
                  if j % 8 == 2 and j >= 10:
                      b = (j - 2) // 8 - 1
                      nc.sync.dma_start(
                          MEAS_d[1][:, b],
                          msb[NMEAS:2 * NMEAS, (b % 2) * 8:(b % 2) * 8 + 8])

              cur, prev, spare = spare, cur, prev

        # post-loop: stage slot nt-1 = p^(nt) via the aug columns of one
        # extra band pair per field (annex/band rows write junk state rows
        # that are never read; aug rows are a pure copy of p^(nt))
        for f in range(BB):
            O = PS[f][nt % 2]
            nc.tensor.matmul(O[0:128, 0, 2:2 + D], ct["BD0"],
                             TT[cur][0:KA, f, 0, 2:2 + D],
                             start=True, stop=False, skip_group_check=True)
            nc.tensor.matmul(O[0:128, 1, 2:2 + D], ct["BD1"],
                             TT[cur][0:KA, f, 1, 2:2 + D],
                             start=True, stop=True, skip_group_check=True)
        for f in range(BB):
            nc.scalar.copy(msb[32 * f:32 * f + NMEAS, (nt - 1) % 16],
                           PS[f][nt % 2][96:96 + NMEAS, 0:2, 2:2 + D])
        # flush the final (partial) block for both fields
        b = (nt - 1) // 8
        for f in range(BB):
            nc.sync.dma_start(
                MEAS_d[f][:, b],
                msb[32 * f:32 * f + NMEAS, (b % 2) * 8:(b % 2) * 8 + 8])

    nc.compile()
    return nc


def kernel(x, P0):
    x = np.asarray(x, dtype=np.float32)
    P0 = np.asarray(P0, dtype=np.float32)
    from concourse.bass_utils import run_bass_kernel_spmd

    if "prog" not in _prog_cache:
        _prog_cache["prog"] = _build_program()
    nc = _prog_cache["prog"]

    cb = _build_band_consts()
    order = ["BD0", "BD1", "TAP1", "TAP2", "XB"]
    wid = sum(cb[n].shape[1] for n in order)
    cp = np.zeros((KA, wid), np.float16)
    co = 0
    for n in order:
        a = cb[n]
        cp[0:a.shape[0], co:co + a.shape[1]] = a
        co += a.shape[1]
    consts = {"CPACK": cp}

    xx = bg / x[:, 0]
    rf = (1.0 - xx * xx).astype(np.float32)           # (B, 96, 96)
    P0c = P0[0, :, :, CLO:CHI, CLO:CHI]               # (NR, NT, 96, 96)
    d2 = np.zeros_like(P0c)
    d2[:, 2:] = P0c[:, 2:] - 2.0 * P0c[:, 1:-1] + P0c[:, :-2]

    grows = np.array([_gpart(s) - 71 for s in range(48)])
    in_maps = []
    for r in range(NRR):
        Gc = (rf[None, :, :, :] * d2[r][:, None, :, :]).astype(np.float16)
        G = np.zeros((NT, 50, BB, 2, SEG), np.float16)
        # fancy index on axis 1 puts that axis first in the result
        G[:, grows, :, 0, 2 + COFF:2 + COFF + CW] = \
            Gc[:, :, 0:48, :].transpose(2, 0, 1, 3)
        G[:, grows, :, 1, 2 + COFF:2 + COFF + CW] = \
            Gc[:, :, 48:96, :].transpose(2, 0, 1, 3)
        m = dict(consts)
        m["G"] = G
        in_maps.append(m)

    trace = bool(int(os.environ.get("KERNEL_TRACE", "0")))
    res = run_bass_kernel_spmd(nc, in_maps, core_ids=list(range(NRR)),
                               trace=trace)
    _prog_cache["last_result"] = res

    ry = _MY - LO
    out = np.zeros((BB, NRR, NMEAS, NT), np.float32)
    nblk = (NT + 7) // 8
    for r in range(NRR):
        Ms = res.results[r]["MEAS"].reshape(BB, NMEAS, nblk * 8, 2, D)
        for f in range(BB):
            for pos, i in enumerate(_ORDER):
                c = 0 if _GROW[i] < K else 1
                out[f, r, i] = Ms[f, pos, :NT, c, ry[i]].astype(np.float32)
            out[f, r, :, 0] = 0.0
    return out
